# revision 69
# baseline (speedup 1.0000x reference)
"""MoE expert-collection grouped GEMM for Trainium2, expert-parallel over 8
NeuronCores.

Problem (hardcoded shapes):
  sorted_features  [65536, 1024] f32   tokens sorted by expert, 4096/expert
  expert_ids_sorted[65536] i32         unused: split is static equal-count
  routing_matrix   [1024, 2048, 16] f32
  routing_bias     [2048, 16] f32
  out = silu(x_e @ W_e + b_e) per expert  -> [65536, 2048] f32

Sharding: expert-parallel, 2 experts (= 8192 contiguous sorted tokens) per
core. Host-side dispatch hands each core its token block transposed
(feature-major, fp8 e4m3) plus its 2 experts' weights (fp8 e4m3, pre-scaled
x128 so w_std 0.0054 lands in e4m3's normal range) and bias pre-broadcast to
128 partitions (fp32, pre-scaled x128 to match).

Device pipeline per core: 1024 fp8 DoubleRow matmuls (K=256 per instruction,
2x PE throughput vs fp16) accumulating in fp32 PSUM (t-on-partitions x
o-free tiles, contraction over 4 k-pair blocks), DVE bias add (in fp32 x128
domain, fp16 out), ACT Silu with scale=1/128 folding the weight scale back
out (fp16 out), fp16 store. x loads ride the SP HWDGE ring; weight loads and
output stores ride the ACT HWDGE ring.
"""

import ml_dtypes
import numpy as np

import concourse.bass as bass
import concourse.mybir as mybir
import concourse.tile as tile
from concourse.bass_utils import run_bass_kernel_spmd

N_CORES = 8
N_TOKENS = 65536
D_IN = 1024
D_OUT = 2048
N_EXPERTS = 16
E_PER_CORE = N_EXPERTS // N_CORES        # 2
TOK_PER_CORE = N_TOKENS // N_CORES       # 8192
TOK_PER_EXPERT = N_TOKENS // N_EXPERTS   # 4096

P = 128
KB = D_IN // P            # 8 contraction blocks
TS = 512                  # token stripe
OB = 512                  # out-feature block (one PSUM bank)
N_OB = D_OUT // OB        # 4
N_TSUB = TS // P          # 4
STRIPES_PER_EXPERT = TOK_PER_EXPERT // TS  # 8

F32 = mybir.dt.float32
F16 = mybir.dt.float16
F8 = mybir.dt.float8e4
NP_F8 = ml_dtypes.float8_e4m3
W_SCALE = 128.0  # lifts w_std ~0.0054 out of e4m3 subnormal territory
KH_G = 2          # kb per W tile = one DoubleRow k-pair
NWT_G = KB // KH_G  # 4 W tiles per expert


def _split_multi_waits(nc):
    """This container's walrus encodes at most ONE sync-wait per instruction;
    hoist extras onto single-wait NoOps inserted just before, same engine."""
    for fn in nc.m.functions:
        for bb in fn.blocks:
            insts = list(bb.instructions)
            out = []
            dirty = False
            for inst in insts:
                si = inst.sync_info
                waits = list(si.on_wait) if si and si.on_wait else []
                if len(waits) > 1:
                    dirty = True
                    for j, w in enumerate(waits[:-1]):
                        nop = mybir.InstNoOp(
                            name=f"{inst.name}-prewait{j}", ins=[], outs=[]
                        )
                        nop.engine = inst.engine
                        nop.sync_info = mybir.SyncInfo(on_wait=[w], on_update=[])
                        out.append(nop)
                    inst.sync_info = mybir.SyncInfo(
                        on_wait=[waits[-1]],
                        on_update=list(si.on_update) if si.on_update else [],
                    )
                out.append(inst)
            if dirty:
                bb.instructions = out


N_STRIPES = E_PER_CORE * STRIPES_PER_EXPERT  # 16


def build_kernel():
    nc = bass.Bass()
    # xt pre-striped on host: [stripe, partition, kb, t] so each stripe loads
    # with 8KB-contiguous per-partition lines
    xt = nc.dram_tensor("xt", [N_STRIPES, P, KB, TS], F8, kind="ExternalInput")
    # w pre-packed on host into the exact sbuf tile layout [e, h, p, os, kh, o']
    # so W DMAs are fully contiguous per-partition reads with 4KB (full-tile)
    # or 2KB (os-half) elements — the naive "(kb p) o" rearrange reads
    # scattered 1-2KB chunks at a fraction of the per-queue bandwidth
    # one pack per expert in sbuf tile layout [p, os, h, oc, kh, o'']: a full
    # os-half (all 4 h tiles, 1MB) is ONE contiguous transfer — cold DMAs
    # cost ~4-5us nearly independent of size, so the ramp wants FEW, LARGE
    # transfers
    w = nc.dram_tensor(
        "w", [E_PER_CORE, P, 2, NWT_G, 2, KH_G, D_OUT // 4], F8,
        kind="ExternalInput",
    )
    # bias pre-broadcast on host in fp16 (512KB/expert; fp32 was 1MB of
    # redundant DMA sitting in front of ramp-critical W slices, and the
    # on-device partition_broadcast op doesn't encode in this toolchain)
    bb = nc.dram_tensor("bb", [E_PER_CORE, P, D_OUT], F16, kind="ExternalInput")
    y = nc.dram_tensor("y", [TOK_PER_CORE, D_OUT], F16, kind="ExternalOutput")

    with tile.TileContext(nc) as tc:
        with (
            tc.tile_pool(name="persist", bufs=1) as persist,
            tc.tile_pool(name="xp", bufs=4) as xp,
            tc.tile_pool(name="outs", bufs=4) as outs,
            tc.tile_pool(name="psum", bufs=8, space="PSUM") as psump,
        ):
            x16_tiles = {}
            x16_tiles[0] = xp.tile([P, KB, TS], F8, tag="x16", name="x16_s0")
            nc.sync.dma_start(x16_tiles[0][:], xt[0])

            KH = KH_G
            NWT = NWT_G
            b_sb = [
                persist.tile([P, D_OUT], F16, name=f"bias_{e}")
                for e in range(E_PER_CORE)
            ]
            # one W tile per expert: [p, os, h, oc, kh, o'']; an os-half is a
            # contiguous 8KB-per-partition run = one 1MB DMA
            w16 = [
                persist.tile(
                    [P, 2, NWT, 2, KH, D_OUT // 4], F8, name=f"w16_{e}"
                )
                for e in range(E_PER_CORE)
            ]

            half = D_OUT // 2

            def load_expert0():
                # expert 0 gates the ramp: os0 arrives as h01 (512KB, cold)
                # then h23 (512KB, warm — same queue warms after the first
                # transfer), so partial-K matmuls can start ~2us earlier and
                # bridge until h23 lands; then os1. Bias first-half rides
                # sync behind x0 (first DVE read comes well after the first
                # matmul, with 8 PSUM banks of runway). gpsimd's ring is ~2x
                # slower when cold, so it only carries the late bias half.
                nc.scalar.dma_start(w16[0][:, 0], w[0, :, 0])
                nc.sync.dma_start(b_sb[0][:, :half], bb[0][:, :half])
                nc.scalar.dma_start(w16[0][:, 1], w[0, :, 1])
                nc.gpsimd.dma_start(b_sb[0][:, half:], bb[0][:, half:])

            def load_expert1():
                # mid-flight on warm queues, off the critical path
                nc.gpsimd.dma_start(b_sb[1][:], bb[1])
                nc.scalar.dma_start(w16[1][:, 0], w[1, :, 0])
                nc.sync.dma_start(w16[1][:, 1], w[1, :, 1])

            load_expert0()

            # PE warmup: dummy DoubleRow matmuls on a memset scratch tile so
            # the tensor engine is at full p-state clock (not the 1.2GHz ramp
            # tier) by the time the critical preload lands; also converts the
            # ~5us data-starved head into busy time
            wu = persist.tile([P, 2, OB], F8, name="warmup")
            nc.vector.memset(wu[:], 0)
            wu_ps = psump.tile([P, OB], F32, tag="ps")
            N_WU = 16
            for i in range(N_WU):
                nc.tensor.matmul(
                    wu_ps[:],
                    lhsT=wu[:, :, 0:P],
                    rhs=wu[:],
                    start=(i == 0),
                    stop=(i == N_WU - 1),
                    perf_mode=mybir.MatmulPerfMode.DoubleRow,
                )

            def mm_part(x16, e, tsub, ob, ps, ha, hb, start, stop):
                os_, oc = divmod(ob, 2)
                for h in range(ha, hb):
                    # DoubleRow: K=256 (one kb pair) per matmul
                    nc.tensor.matmul(
                        ps[:],
                        lhsT=x16[:, 2 * h:2 * h + 2, tsub * P:(tsub + 1) * P],
                        rhs=w16[e][:, os_, h, oc],
                        start=(start and h == ha),
                        stop=(stop and h == hb - 1),
                        perf_mode=mybir.MatmulPerfMode.DoubleRow,
                    )

            def mm_group(x16, e, tsub, ob, ps):
                mm_part(x16, e, tsub, ob, ps, 0, NWT, True, True)

            # stripe 0, ob-major: all os0 groups first so the PE ramp only
            # waits on the first half of expert-0's weights; per-half silu +
            # store keeps downstream engines streaming during the ramp
            x0 = x16_tiles[0]
            for os_ in range(2):
                for tsub in range(N_TSUB):
                    yp = outs.tile([P, half], F16, tag="ypreh")
                    ya = outs.tile([P, half], F16, tag="yacth")
                    for oc in range(2):
                        ob = os_ * 2 + oc
                        ps = psump.tile([P, OB], F32, tag="ps")
                        mm_group(x0, 0, tsub, ob, ps)
                        nc.vector.tensor_tensor(
                            yp[:, oc * OB:(oc + 1) * OB], ps[:],
                            b_sb[0][:, ob * OB:(ob + 1) * OB],
                            mybir.AluOpType.add,
                        )
                    nc.scalar.activation(
                        ya[:], yp[:],
                        mybir.ActivationFunctionType.Silu,
                        scale=1.0 / W_SCALE,
                    )
                    [nc.gpsimd, nc.sync, nc.scalar][
                        (os_ * N_TSUB + tsub) % 3
                    ].dma_start(
                        y[tsub * P:(tsub + 1) * P,
                          os_ * half:(os_ + 1) * half],
                        ya[:],
                    )

            for e in range(E_PER_CORE):
                for s in range(STRIPES_PER_EXPERT):
                    g = e * STRIPES_PER_EXPERT + s
                    if g == 0:
                        continue  # handled above, ob-major
                    t0 = g * TS
                    x16 = xp.tile([P, KB, TS], F8, tag="x16", name="x16")
                    nc.sync.dma_start(x16[:], xt[g])

                    if g == N_STRIPES - 1:
                        # final stripe: per os-half silu + 256KB stores (last
                        # tsub: per-quarter) spread over the sync/scalar
                        # rings — gpsimd's slow drain stays off the tail
                        for tsub in range(N_TSUB):
                            if tsub == N_TSUB - 1:
                                # very last tile: 256-wide DVE/ACT/store
                                # chunks so the post-matmul chain is as
                                # short as possible
                                for ob in range(N_OB):
                                    yq = outs.tile([P, OB], F16, tag="yq")
                                    ya = outs.tile([P, OB], F16, tag="yaq")
                                    ps = psump.tile([P, OB], F32, tag="ps")
                                    mm_group(x16, e, tsub, ob, ps)
                                    fine = ob == N_OB - 1
                                    QW = OB // 2 if fine else OB
                                    for qc in range(OB // QW):
                                        sl = slice(qc * QW, (qc + 1) * QW)
                                        nc.vector.tensor_tensor(
                                            yq[:, sl], ps[:, sl],
                                            b_sb[e][:, ob * OB + qc * QW:
                                                    ob * OB + (qc + 1) * QW],
                                            mybir.AluOpType.add,
                                        )
                                        nc.scalar.activation(
                                            ya[:, sl], yq[:, sl],
                                            mybir.ActivationFunctionType.Silu,
                                            scale=1.0 / W_SCALE,
                                        )
                                        [nc.sync, nc.scalar][
                                            (ob + qc) % 2
                                        ].dma_start(
                                            y[t0 + tsub * P:
                                              t0 + (tsub + 1) * P,
                                              ob * OB + qc * QW:
                                              ob * OB + (qc + 1) * QW],
                                            ya[:, sl],
                                        )
                                continue
                            for os_ in range(2):
                                yp = outs.tile([P, half], F16, tag="ypreh")
                                ya = outs.tile([P, half], F16, tag="yacth")
                                for oc in range(2):
                                    ob = os_ * 2 + oc
                                    ps = psump.tile([P, OB], F32, tag="ps")
                                    mm_group(x16, e, tsub, ob, ps)
                                    nc.vector.tensor_tensor(
                                        yp[:, oc * OB:(oc + 1) * OB], ps[:],
                                        b_sb[e][:, ob * OB:(ob + 1) * OB],
                                        mybir.AluOpType.add,
                                    )
                                nc.scalar.activation(
                                    ya[:], yp[:],
                                    mybir.ActivationFunctionType.Silu,
                                    scale=1.0 / W_SCALE,
                                )
                                [nc.sync, nc.scalar][
                                    (tsub * 2 + os_) % 2
                                ].dma_start(
                                    y[t0 + tsub * P:t0 + (tsub + 1) * P,
                                      os_ * half:(os_ + 1) * half],
                                    ya[:],
                                )
                        continue

                    for tsub in range(N_TSUB):
                        store_eng = [nc.gpsimd, nc.sync, nc.scalar][
                            (g * N_TSUB + tsub) % 3
                        ]
                        y_pre = outs.tile([P, D_OUT], F16, tag="ypre")
                        y_act = outs.tile([P, D_OUT], F16, tag="yact")
                        for ob in range(N_OB):
                            ps = psump.tile([P, OB], F32, tag="ps")
                            mm_group(x16, e, tsub, ob, ps)
                            # bias add in the x128 domain (bias pre-scaled on
                            # host); fp16 out is exact enough at |v|<~700
                            nc.vector.tensor_tensor(
                                y_pre[:, ob * OB:(ob + 1) * OB], ps[:],
                                b_sb[e][:, ob * OB:(ob + 1) * OB],
                                mybir.AluOpType.add,
                            )
                        # one fused silu per 2048-wide tile amortizes the
                        # ~300ns fixed ACT cost; scale folds the x128
                        # weight scale back out before the nonlinearity
                        nc.scalar.activation(
                            y_act[:], y_pre[:],
                            mybir.ActivationFunctionType.Silu,
                            scale=1.0 / W_SCALE,
                        )
                        store_eng.dma_start(
                            y[t0 + tsub * P:t0 + (tsub + 1) * P, :], y_act[:]
                        )
                    if g == 1:
                        # after g==1 so expert 1's 1MB on the sync ring sits
                        # behind the already-enqueued x1/x2 prefetches
                        load_expert1()

    _split_multi_waits(nc)
    return nc


_NC_CACHE = None


def _get_nc():
    global _NC_CACHE
    if _NC_CACHE is None:
        _NC_CACHE = build_kernel()
    return _NC_CACHE


def _in_maps(sorted_features, routing_matrix, routing_bias):
    maps = []
    for c in range(N_CORES):
        rows = slice(c * TOK_PER_CORE, (c + 1) * TOK_PER_CORE)
        es = slice(c * E_PER_CORE, (c + 1) * E_PER_CORE)
        # [stripe, partition, kb, t]: element (s,p,kb,t) = X_c[s*TS+t, kb*P+p]
        xt_c = np.ascontiguousarray(
            sorted_features[rows]
            .reshape(N_STRIPES, TS, KB, P)
            .transpose(0, 3, 2, 1)
            .astype(NP_F8)
        )
        # pack into the device tile layout [e, p, os, h, oc, kh, o'']:
        # kin = (h*KH+kh)*128 + p, o = os*1024 + oc*512 + o''
        w_c = np.ascontiguousarray(
            (routing_matrix[:, :, es].transpose(2, 0, 1) * W_SCALE)
            .astype(NP_F8)
            .reshape(E_PER_CORE, NWT_G, KH_G, P, 2, 2, D_OUT // 4)
            .transpose(0, 3, 4, 1, 5, 2, 6)
        )
        # bias enters the DVE add in the x128 domain: silu((ps + S*b)/S);
        # fp16 is exact to ~2^-11 relative, far under the fp8 matmul noise
        b_c = np.ascontiguousarray(
            np.broadcast_to(
                (routing_bias[:, es].T * W_SCALE)[:, None, :],
                (E_PER_CORE, P, D_OUT),
            ).astype(np.float16)
        )
        maps.append({"xt": xt_c, "w": w_c, "bb": b_c})
    return maps


def run(sorted_features, routing_matrix, routing_bias, **run_kwargs):
    nc = _get_nc()
    maps = _in_maps(sorted_features, routing_matrix, routing_bias)
    res = run_bass_kernel_spmd(nc, maps, core_ids=list(range(N_CORES)), **run_kwargs)
    out = np.concatenate(
        [res.results[c]["y"].astype(np.float32) for c in range(N_CORES)], axis=0
    )
    return out, res


def kernel(sorted_features, expert_ids_sorted, routing_matrix, routing_bias):
    assert sorted_features.shape == (N_TOKENS, D_IN)
    assert routing_matrix.shape == (D_IN, D_OUT, N_EXPERTS)
    assert routing_bias.shape == (D_OUT, N_EXPERTS)
    out, _ = run(
        np.asarray(sorted_features, dtype=np.float32),
        np.asarray(routing_matrix, dtype=np.float32),
        np.asarray(routing_bias, dtype=np.float32),
    )
    return out



# revision 78
# speedup vs baseline: 1.0061x; 1.0061x over previous
"""MoE expert-collection grouped GEMM for Trainium2, expert-parallel over 8
NeuronCores.

Problem (hardcoded shapes):
  sorted_features  [65536, 1024] f32   tokens sorted by expert, 4096/expert
  expert_ids_sorted[65536] i32         unused: split is static equal-count
  routing_matrix   [1024, 2048, 16] f32
  routing_bias     [2048, 16] f32
  out = silu(x_e @ W_e + b_e) per expert  -> [65536, 2048] f32

Sharding: expert-parallel, 2 experts (= 8192 contiguous sorted tokens) per
core. Host-side dispatch hands each core its token block transposed
(feature-major, fp8 e4m3) plus its 2 experts' weights (fp8 e4m3, pre-scaled
x128 so w_std 0.0054 lands in e4m3's normal range) and bias pre-broadcast to
128 partitions (fp32, pre-scaled x128 to match).

Device pipeline per core: 1024 fp8 DoubleRow matmuls (K=256 per instruction,
2x PE throughput vs fp16) accumulating in fp32 PSUM (t-on-partitions x
o-free tiles, contraction over 4 k-pair blocks), DVE bias add (in fp32 x128
domain, fp16 out), ACT Silu with scale=1/128 folding the weight scale back
out (fp16 out), fp16 store. x loads ride the SP HWDGE ring; weight loads and
output stores ride the ACT HWDGE ring.
"""

import ml_dtypes
import numpy as np

import concourse.bass as bass
import concourse.mybir as mybir
import concourse.tile as tile
from concourse.bass_utils import run_bass_kernel_spmd

N_CORES = 8
N_TOKENS = 65536
D_IN = 1024
D_OUT = 2048
N_EXPERTS = 16
E_PER_CORE = N_EXPERTS // N_CORES        # 2
TOK_PER_CORE = N_TOKENS // N_CORES       # 8192
TOK_PER_EXPERT = N_TOKENS // N_EXPERTS   # 4096

P = 128
KB = D_IN // P            # 8 contraction blocks
TS = 512                  # token stripe
OB = 512                  # out-feature block (one PSUM bank)
N_OB = D_OUT // OB        # 4
N_TSUB = TS // P          # 4
STRIPES_PER_EXPERT = TOK_PER_EXPERT // TS  # 8

F32 = mybir.dt.float32
F16 = mybir.dt.float16
F8 = mybir.dt.float8e4
NP_F8 = ml_dtypes.float8_e4m3
W_SCALE = 128.0  # lifts w_std ~0.0054 out of e4m3 subnormal territory
KH_G = 2          # kb per W tile = one DoubleRow k-pair
NWT_G = KB // KH_G  # 4 W tiles per expert


def _split_multi_waits(nc):
    """This container's walrus encodes at most ONE sync-wait per instruction;
    hoist extras onto single-wait NoOps inserted just before, same engine."""
    for fn in nc.m.functions:
        for bb in fn.blocks:
            insts = list(bb.instructions)
            out = []
            dirty = False
            for inst in insts:
                si = inst.sync_info
                waits = list(si.on_wait) if si and si.on_wait else []
                if len(waits) > 1:
                    dirty = True
                    for j, w in enumerate(waits[:-1]):
                        nop = mybir.InstNoOp(
                            name=f"{inst.name}-prewait{j}", ins=[], outs=[]
                        )
                        nop.engine = inst.engine
                        nop.sync_info = mybir.SyncInfo(on_wait=[w], on_update=[])
                        out.append(nop)
                    inst.sync_info = mybir.SyncInfo(
                        on_wait=[waits[-1]],
                        on_update=list(si.on_update) if si.on_update else [],
                    )
                out.append(inst)
            if dirty:
                bb.instructions = out


N_STRIPES = E_PER_CORE * STRIPES_PER_EXPERT  # 16


def build_kernel():
    nc = bass.Bass()
    # xt pre-striped on host: [stripe, partition, kb, t] so each stripe loads
    # with 8KB-contiguous per-partition lines
    xt = nc.dram_tensor("xt", [N_STRIPES, P, KB, TS], F8, kind="ExternalInput")
    # w pre-packed on host into the exact sbuf tile layout [e, h, p, os, kh, o']
    # so W DMAs are fully contiguous per-partition reads with 4KB (full-tile)
    # or 2KB (os-half) elements — the naive "(kb p) o" rearrange reads
    # scattered 1-2KB chunks at a fraction of the per-queue bandwidth
    # one pack per expert in sbuf tile layout [p, os, h, oc, kh, o'']: a full
    # os-half (all 4 h tiles, 1MB) is ONE contiguous transfer — cold DMAs
    # cost ~4-5us nearly independent of size, so the ramp wants FEW, LARGE
    # transfers
    w = nc.dram_tensor(
        "w", [E_PER_CORE, P, 2, NWT_G, 2, KH_G, D_OUT // 4], F8,
        kind="ExternalInput",
    )
    # bias pre-broadcast on host in fp16 (512KB/expert; fp32 was 1MB of
    # redundant DMA sitting in front of ramp-critical W slices, and the
    # on-device partition_broadcast op doesn't encode in this toolchain)
    bb = nc.dram_tensor("bb", [E_PER_CORE, P, D_OUT], F16, kind="ExternalInput")
    y = nc.dram_tensor("y", [TOK_PER_CORE, D_OUT], F16, kind="ExternalOutput")

    with tile.TileContext(nc) as tc:
        with (
            tc.tile_pool(name="persist", bufs=1) as persist,
            tc.tile_pool(name="xp", bufs=4) as xp,
            tc.tile_pool(name="outs", bufs=4) as outs,
            tc.tile_pool(name="psum", bufs=8, space="PSUM") as psump,
        ):
            x16_tiles = {}
            x16_tiles[0] = xp.tile([P, KB, TS], F8, tag="x16", name="x16_s0")
            nc.sync.dma_start(x16_tiles[0][:], xt[0])

            KH = KH_G
            NWT = NWT_G
            b_sb = [
                persist.tile([P, D_OUT], F16, name=f"bias_{e}")
                for e in range(E_PER_CORE)
            ]
            # one W tile per expert: [p, os, h, oc, kh, o'']; an os-half is a
            # contiguous 8KB-per-partition run = one 1MB DMA
            w16 = [
                persist.tile(
                    [P, 2, NWT, 2, KH, D_OUT // 4], F8, name=f"w16_{e}"
                )
                for e in range(E_PER_CORE)
            ]

            half = D_OUT // 2

            def load_expert0():
                # expert 0 gates the ramp: os0 arrives as h01 (512KB, cold)
                # then h23 (512KB, warm — same queue warms after the first
                # transfer), so partial-K matmuls can start ~2us earlier and
                # bridge until h23 lands; then os1. Bias first-half rides
                # sync behind x0 (first DVE read comes well after the first
                # matmul, with 8 PSUM banks of runway). gpsimd's ring is ~2x
                # slower when cold, so it only carries the late bias half.
                nc.scalar.dma_start(w16[0][:, 0], w[0, :, 0])
                nc.sync.dma_start(b_sb[0][:, :half], bb[0][:, :half])
                nc.scalar.dma_start(w16[0][:, 1], w[0, :, 1])
                nc.gpsimd.dma_start(b_sb[0][:, half:], bb[0][:, half:])

            def load_expert1():
                # mid-flight on warm queues, off the critical path
                nc.gpsimd.dma_start(b_sb[1][:], bb[1])
                nc.scalar.dma_start(w16[1][:, 0], w[1, :, 0])
                nc.sync.dma_start(w16[1][:, 1], w[1, :, 1])

            load_expert0()

            # PE warmup: dummy DoubleRow matmuls on a memset scratch tile so
            # the tensor engine is at full p-state clock (not the 1.2GHz ramp
            # tier) by the time the critical preload lands; also converts the
            # ~5us data-starved head into busy time
            wu = persist.tile([P, 2, OB], F8, name="warmup")
            nc.vector.memset(wu[:], 0)
            wu_ps = psump.tile([P, OB], F32, tag="ps")
            N_WU = 15
            for i in range(N_WU):
                nc.tensor.matmul(
                    wu_ps[:],
                    lhsT=wu[:, :, 0:P],
                    rhs=wu[:],
                    start=(i == 0),
                    stop=(i == N_WU - 1),
                    perf_mode=mybir.MatmulPerfMode.DoubleRow,
                )

            def mm_group(x16, e, tsub, ob, ps_ap):
                os_, oc = divmod(ob, 2)
                for h in range(NWT):
                    # DoubleRow: K=256 (one kb pair) per matmul
                    nc.tensor.matmul(
                        ps_ap,
                        lhsT=x16[:, 2 * h:2 * h + 2, tsub * P:(tsub + 1) * P],
                        rhs=w16[e][:, os_, h, oc],
                        start=(h == 0),
                        stop=(h == NWT - 1),
                        perf_mode=mybir.MatmulPerfMode.DoubleRow,
                    )

            # stripe 0, ob-major: all os0 groups first so the PE ramp only
            # waits on the first half of expert-0's weights; per-half silu +
            # store keeps downstream engines streaming during the ramp
            x0 = x16_tiles[0]
            for os_ in range(2):
                for tsub in range(N_TSUB):
                    yp = outs.tile([P, half], F16, tag="ypreh")
                    ya = outs.tile([P, half], F16, tag="yacth")
                    for oc in range(2):
                        ob = os_ * 2 + oc
                        ps = psump.tile([P, OB], F32, tag="ps")
                        mm_group(x0, 0, tsub, ob, ps[:])
                        nc.vector.tensor_tensor(
                            yp[:, oc * OB:(oc + 1) * OB], ps[:],
                            b_sb[0][:, ob * OB:(ob + 1) * OB],
                            mybir.AluOpType.add,
                        )
                    nc.scalar.activation(
                        ya[:], yp[:],
                        mybir.ActivationFunctionType.Silu,
                        scale=1.0 / W_SCALE,
                    )
                    [nc.gpsimd, nc.sync, nc.scalar][
                        (os_ * N_TSUB + tsub) % 3
                    ].dma_start(
                        y[tsub * P:(tsub + 1) * P,
                          os_ * half:(os_ + 1) * half],
                        ya[:],
                    )

            for e in range(E_PER_CORE):
                for s in range(STRIPES_PER_EXPERT):
                    g = e * STRIPES_PER_EXPERT + s
                    if g == 0:
                        continue  # handled above, ob-major
                    t0 = g * TS
                    x16 = xp.tile([P, KB, TS], F8, tag="x16", name="x16")
                    nc.sync.dma_start(x16[:], xt[g])

                    if g == N_STRIPES - 1:
                        # final stripe: per os-half silu + 256KB stores (last
                        # tsub: per-quarter) spread over the sync/scalar
                        # rings — gpsimd's slow drain stays off the tail
                        for tsub in range(N_TSUB):
                            fine = tsub == N_TSUB - 1
                            for os_ in range(2):
                                yp = outs.tile([P, half], F16, tag="ypreh")
                                ya = outs.tile([P, half], F16, tag="yacth")
                                for oc in range(2):
                                    ob = os_ * 2 + oc
                                    ps = psump.tile([P, OB], F32, tag="ps")
                                    mm_group(x16, e, tsub, ob, ps[:])
                                    sl = slice(oc * OB, (oc + 1) * OB)
                                    nc.vector.tensor_tensor(
                                        yp[:, sl], ps[:],
                                        b_sb[e][:, ob * OB:(ob + 1) * OB],
                                        mybir.AluOpType.add,
                                    )
                                    if fine:
                                        # very last tile: 512-wide chunks so
                                        # the post-matmul chain stays short
                                        nc.scalar.activation(
                                            ya[:, sl], yp[:, sl],
                                            mybir.ActivationFunctionType.Silu,
                                            scale=1.0 / W_SCALE,
                                        )
                                        [nc.sync, nc.scalar][
                                            (os_ + oc) % 2
                                        ].dma_start(
                                            y[t0 + tsub * P:
                                              t0 + (tsub + 1) * P,
                                              ob * OB:(ob + 1) * OB],
                                            ya[:, sl],
                                        )
                                if fine:
                                    continue
                                nc.scalar.activation(
                                    ya[:], yp[:],
                                    mybir.ActivationFunctionType.Silu,
                                    scale=1.0 / W_SCALE,
                                )
                                [nc.sync, nc.scalar][
                                    (tsub * 2 + os_) % 2
                                ].dma_start(
                                    y[t0 + tsub * P:t0 + (tsub + 1) * P,
                                      os_ * half:(os_ + 1) * half],
                                    ya[:],
                                )
                        continue

                    for tsub in range(N_TSUB):
                        store_eng = [nc.gpsimd, nc.sync, nc.scalar][
                            (g * N_TSUB + tsub) % 3
                        ]
                        y_pre = outs.tile([P, D_OUT], F16, tag="ypre")
                        y_act = outs.tile([P, D_OUT], F16, tag="yact")
                        for ob in range(N_OB):
                            ps = psump.tile([P, OB], F32, tag="ps")
                            mm_group(x16, e, tsub, ob, ps[:])
                            # bias add in the x128 domain (bias pre-scaled on
                            # host); fp16 out is exact enough at |v|<~700
                            nc.vector.tensor_tensor(
                                y_pre[:, ob * OB:(ob + 1) * OB], ps[:],
                                b_sb[e][:, ob * OB:(ob + 1) * OB],
                                mybir.AluOpType.add,
                            )
                        # one fused silu per 2048-wide tile amortizes the
                        # ~300ns fixed ACT cost; scale folds the x128
                        # weight scale back out before the nonlinearity
                        nc.scalar.activation(
                            y_act[:], y_pre[:],
                            mybir.ActivationFunctionType.Silu,
                            scale=1.0 / W_SCALE,
                        )
                        store_eng.dma_start(
                            y[t0 + tsub * P:t0 + (tsub + 1) * P, :], y_act[:]
                        )
                    if g == 1:
                        # after g==1 so expert 1's 1MB on the sync ring sits
                        # behind the already-enqueued x1/x2 prefetches
                        load_expert1()

    _split_multi_waits(nc)
    return nc


_NC_CACHE = None


def _get_nc():
    global _NC_CACHE
    if _NC_CACHE is None:
        _NC_CACHE = build_kernel()
    return _NC_CACHE


def _in_maps(sorted_features, routing_matrix, routing_bias):
    maps = []
    for c in range(N_CORES):
        rows = slice(c * TOK_PER_CORE, (c + 1) * TOK_PER_CORE)
        es = slice(c * E_PER_CORE, (c + 1) * E_PER_CORE)
        # [stripe, partition, kb, t]: element (s,p,kb,t) = X_c[s*TS+t, kb*P+p]
        xt_c = np.ascontiguousarray(
            sorted_features[rows]
            .reshape(N_STRIPES, TS, KB, P)
            .transpose(0, 3, 2, 1)
            .astype(NP_F8)
        )
        # pack into the device tile layout [e, p, os, h, oc, kh, o'']:
        # kin = (h*KH+kh)*128 + p, o = os*1024 + oc*512 + o''
        w_c = np.ascontiguousarray(
            (routing_matrix[:, :, es].transpose(2, 0, 1) * W_SCALE)
            .astype(NP_F8)
            .reshape(E_PER_CORE, NWT_G, KH_G, P, 2, 2, D_OUT // 4)
            .transpose(0, 3, 4, 1, 5, 2, 6)
        )
        # bias enters the DVE add in the x128 domain: silu((ps + S*b)/S);
        # fp16 is exact to ~2^-11 relative, far under the fp8 matmul noise
        b_c = np.ascontiguousarray(
            np.broadcast_to(
                (routing_bias[:, es].T * W_SCALE)[:, None, :],
                (E_PER_CORE, P, D_OUT),
            ).astype(np.float16)
        )
        maps.append({"xt": xt_c, "w": w_c, "bb": b_c})
    return maps


def run(sorted_features, routing_matrix, routing_bias, **run_kwargs):
    nc = _get_nc()
    maps = _in_maps(sorted_features, routing_matrix, routing_bias)
    res = run_bass_kernel_spmd(nc, maps, core_ids=list(range(N_CORES)), **run_kwargs)
    out = np.concatenate(
        [res.results[c]["y"].astype(np.float32) for c in range(N_CORES)], axis=0
    )
    return out, res


def kernel(sorted_features, expert_ids_sorted, routing_matrix, routing_bias):
    assert sorted_features.shape == (N_TOKENS, D_IN)
    assert routing_matrix.shape == (D_IN, D_OUT, N_EXPERTS)
    assert routing_bias.shape == (D_OUT, N_EXPERTS)
    out, _ = run(
        np.asarray(sorted_features, dtype=np.float32),
        np.asarray(routing_matrix, dtype=np.float32),
        np.asarray(routing_bias, dtype=np.float32),
    )
    return out



# revision 80
# speedup vs baseline: 1.0061x; 1.0001x over previous
"""MoE expert-collection grouped GEMM for Trainium2, expert-parallel over 8
NeuronCores.

Problem (hardcoded shapes):
  sorted_features  [65536, 1024] f32   tokens sorted by expert, 4096/expert
  expert_ids_sorted[65536] i32         unused: split is static equal-count
  routing_matrix   [1024, 2048, 16] f32
  routing_bias     [2048, 16] f32
  out = silu(x_e @ W_e + b_e) per expert  -> [65536, 2048] f32

Sharding: expert-parallel, 2 experts (= 8192 contiguous sorted tokens) per
core. Host-side dispatch hands each core its token block transposed
(feature-major, fp8 e4m3) plus its 2 experts' weights (fp8 e4m3, pre-scaled
x128 so w_std 0.0054 lands in e4m3's normal range, pre-packed into the sbuf
tile layout for fully contiguous DMA) and bias pre-broadcast to 128
partitions (fp16, pre-scaled x128 to match).

Device pipeline per core: 1024 fp8 DoubleRow matmuls (K=256 per instruction,
2x PE throughput vs fp16) accumulating in fp32 PSUM (t-on-partitions x
o-free tiles, contraction over 4 k-pair blocks), DVE bias add (in the x128
domain, fp16 out), ACT Silu with scale=1/128 folding the weight scale back
out (fp16 out), fp16 stores round-robined over the sync/scalar/gpsimd rings.
PE warmup matmuls cover the NEFF preamble + cold-DMA ramp; measured ~242us
(fp16 baseline 481us), matmul window gapless at the 221us fp8 HW floor.
"""

import ml_dtypes
import numpy as np

import concourse.bass as bass
import concourse.mybir as mybir
import concourse.tile as tile
from concourse.bass_utils import run_bass_kernel_spmd

N_CORES = 8
N_TOKENS = 65536
D_IN = 1024
D_OUT = 2048
N_EXPERTS = 16
E_PER_CORE = N_EXPERTS // N_CORES        # 2
TOK_PER_CORE = N_TOKENS // N_CORES       # 8192
TOK_PER_EXPERT = N_TOKENS // N_EXPERTS   # 4096

P = 128
KB = D_IN // P            # 8 contraction blocks
TS = 512                  # token stripe
OB = 512                  # out-feature block (one PSUM bank)
N_OB = D_OUT // OB        # 4
N_TSUB = TS // P          # 4
STRIPES_PER_EXPERT = TOK_PER_EXPERT // TS  # 8

F32 = mybir.dt.float32
F16 = mybir.dt.float16
F8 = mybir.dt.float8e4
NP_F8 = ml_dtypes.float8_e4m3
W_SCALE = 128.0  # lifts w_std ~0.0054 out of e4m3 subnormal territory
KH_G = 2          # kb per W tile = one DoubleRow k-pair
NWT_G = KB // KH_G  # 4 W tiles per expert


def _split_multi_waits(nc):
    """This container's walrus encodes at most ONE sync-wait per instruction;
    hoist extras onto single-wait NoOps inserted just before, same engine."""
    for fn in nc.m.functions:
        for bb in fn.blocks:
            insts = list(bb.instructions)
            out = []
            dirty = False
            for inst in insts:
                si = inst.sync_info
                waits = list(si.on_wait) if si and si.on_wait else []
                if len(waits) > 1:
                    dirty = True
                    for j, w in enumerate(waits[:-1]):
                        nop = mybir.InstNoOp(
                            name=f"{inst.name}-prewait{j}", ins=[], outs=[]
                        )
                        nop.engine = inst.engine
                        nop.sync_info = mybir.SyncInfo(on_wait=[w], on_update=[])
                        out.append(nop)
                    inst.sync_info = mybir.SyncInfo(
                        on_wait=[waits[-1]],
                        on_update=list(si.on_update) if si.on_update else [],
                    )
                out.append(inst)
            if dirty:
                bb.instructions = out


N_STRIPES = E_PER_CORE * STRIPES_PER_EXPERT  # 16


def build_kernel():
    nc = bass.Bass()
    # xt pre-striped on host: [stripe, partition, kb, t] so each stripe loads
    # with 8KB-contiguous per-partition lines
    xt = nc.dram_tensor("xt", [N_STRIPES, P, KB, TS], F8, kind="ExternalInput")
    # w pre-packed on host into the exact sbuf tile layout [e, h, p, os, kh, o']
    # so W DMAs are fully contiguous per-partition reads with 4KB (full-tile)
    # or 2KB (os-half) elements — the naive "(kb p) o" rearrange reads
    # scattered 1-2KB chunks at a fraction of the per-queue bandwidth
    # one pack per expert in sbuf tile layout [p, os, h, oc, kh, o'']: a full
    # os-half (all 4 h tiles, 1MB) is ONE contiguous transfer — cold DMAs
    # cost ~4-5us nearly independent of size, so the ramp wants FEW, LARGE
    # transfers
    w = nc.dram_tensor(
        "w", [E_PER_CORE, P, 2, NWT_G, 2, KH_G, D_OUT // 4], F8,
        kind="ExternalInput",
    )
    # bias pre-broadcast on host in fp16 (512KB/expert; fp32 was 1MB of
    # redundant DMA sitting in front of ramp-critical W slices, and the
    # on-device partition_broadcast op doesn't encode in this toolchain)
    bb = nc.dram_tensor("bb", [E_PER_CORE, P, D_OUT], F16, kind="ExternalInput")
    y = nc.dram_tensor("y", [TOK_PER_CORE, D_OUT], F16, kind="ExternalOutput")

    with tile.TileContext(nc) as tc:
        with (
            tc.tile_pool(name="persist", bufs=1) as persist,
            tc.tile_pool(name="xp", bufs=4) as xp,
            tc.tile_pool(name="outs", bufs=4) as outs,
            tc.tile_pool(name="psum", bufs=8, space="PSUM") as psump,
        ):
            x16_tiles = {}
            x16_tiles[0] = xp.tile([P, KB, TS], F8, tag="x16", name="x16_s0")
            nc.sync.dma_start(x16_tiles[0][:], xt[0])

            KH = KH_G
            NWT = NWT_G
            b_sb = [
                persist.tile([P, D_OUT], F16, name=f"bias_{e}")
                for e in range(E_PER_CORE)
            ]
            # one W tile per expert: [p, os, h, oc, kh, o'']; an os-half is a
            # contiguous 8KB-per-partition run = one 1MB DMA
            w16 = [
                persist.tile(
                    [P, 2, NWT, 2, KH, D_OUT // 4], F8, name=f"w16_{e}"
                )
                for e in range(E_PER_CORE)
            ]

            half = D_OUT // 2

            def load_expert0():
                # expert 0 gates the ramp, and cold transfers cost ~4-6us
                # nearly independent of size: deliver its whole os0 (1MB —
                # what the first 8 groups need) as ONE scalar-ring transfer,
                # then os1. Bias first-half rides sync behind x0 (first DVE
                # read comes well after the first matmul, with 8 PSUM banks
                # of runway). gpsimd's ring is ~2x slower when cold, so it
                # only carries the late bias half.
                nc.scalar.dma_start(w16[0][:, 0], w[0, :, 0])
                nc.sync.dma_start(b_sb[0][:, :half], bb[0][:, :half])
                nc.scalar.dma_start(w16[0][:, 1], w[0, :, 1])
                nc.gpsimd.dma_start(b_sb[0][:, half:], bb[0][:, half:])

            def load_expert1():
                # mid-flight on warm queues, off the critical path
                nc.gpsimd.dma_start(b_sb[1][:], bb[1])
                nc.scalar.dma_start(w16[1][:, 0], w[1, :, 0])
                nc.sync.dma_start(w16[1][:, 1], w[1, :, 1])

            load_expert0()

            # PE warmup: dummy DoubleRow matmuls on a memset scratch tile so
            # the tensor engine is at full p-state clock (not the 1.2GHz ramp
            # tier) by the time the critical preload lands; also converts the
            # ~5us data-starved head into busy time
            wu = persist.tile([P, 2, OB], F8, name="warmup")
            nc.vector.memset(wu[:], 0)
            wu_ps = psump.tile([P, OB], F32, tag="ps")
            N_WU = 15
            for i in range(N_WU):
                nc.tensor.matmul(
                    wu_ps[:],
                    lhsT=wu[:, :, 0:P],
                    rhs=wu[:],
                    start=(i == 0),
                    stop=(i == N_WU - 1),
                    perf_mode=mybir.MatmulPerfMode.DoubleRow,
                )

            def mm_group(x16, e, tsub, ob, ps_ap):
                os_, oc = divmod(ob, 2)
                for h in range(NWT):
                    # DoubleRow: K=256 (one kb pair) per matmul
                    nc.tensor.matmul(
                        ps_ap,
                        lhsT=x16[:, 2 * h:2 * h + 2, tsub * P:(tsub + 1) * P],
                        rhs=w16[e][:, os_, h, oc],
                        start=(h == 0),
                        stop=(h == NWT - 1),
                        perf_mode=mybir.MatmulPerfMode.DoubleRow,
                    )

            # stripe 0, ob-major: all os0 groups first so the PE ramp only
            # waits on the first half of expert-0's weights; per-half silu +
            # store keeps downstream engines streaming during the ramp
            x0 = x16_tiles[0]
            for os_ in range(2):
                for tsub in range(N_TSUB):
                    yp = outs.tile([P, half], F16, tag="ypreh")
                    ya = outs.tile([P, half], F16, tag="yacth")
                    for oc in range(2):
                        ob = os_ * 2 + oc
                        ps = psump.tile([P, OB], F32, tag="ps")
                        mm_group(x0, 0, tsub, ob, ps[:])
                        nc.vector.tensor_tensor(
                            yp[:, oc * OB:(oc + 1) * OB], ps[:],
                            b_sb[0][:, ob * OB:(ob + 1) * OB],
                            mybir.AluOpType.add,
                        )
                    nc.scalar.activation(
                        ya[:], yp[:],
                        mybir.ActivationFunctionType.Silu,
                        scale=1.0 / W_SCALE,
                    )
                    [nc.gpsimd, nc.sync, nc.scalar][
                        (os_ * N_TSUB + tsub) % 3
                    ].dma_start(
                        y[tsub * P:(tsub + 1) * P,
                          os_ * half:(os_ + 1) * half],
                        ya[:],
                    )

            for e in range(E_PER_CORE):
                for s in range(STRIPES_PER_EXPERT):
                    g = e * STRIPES_PER_EXPERT + s
                    if g == 0:
                        continue  # handled above, ob-major
                    t0 = g * TS
                    x16 = xp.tile([P, KB, TS], F8, tag="x16", name="x16")
                    nc.sync.dma_start(x16[:], xt[g])

                    if g == N_STRIPES - 1:
                        # final stripe: per os-half silu + 256KB stores (last
                        # tsub: per-quarter) spread over the sync/scalar
                        # rings — gpsimd's slow drain stays off the tail
                        for tsub in range(N_TSUB):
                            fine = tsub == N_TSUB - 1
                            for os_ in range(2):
                                yp = outs.tile([P, half], F16, tag="ypreh")
                                ya = outs.tile([P, half], F16, tag="yacth")
                                for oc in range(2):
                                    ob = os_ * 2 + oc
                                    ps = psump.tile([P, OB], F32, tag="ps")
                                    mm_group(x16, e, tsub, ob, ps[:])
                                    sl = slice(oc * OB, (oc + 1) * OB)
                                    nc.vector.tensor_tensor(
                                        yp[:, sl], ps[:],
                                        b_sb[e][:, ob * OB:(ob + 1) * OB],
                                        mybir.AluOpType.add,
                                    )
                                    if fine:
                                        # very last tile: 512-wide chunks so
                                        # the post-matmul chain stays short
                                        nc.scalar.activation(
                                            ya[:, sl], yp[:, sl],
                                            mybir.ActivationFunctionType.Silu,
                                            scale=1.0 / W_SCALE,
                                        )
                                        [nc.sync, nc.scalar][
                                            (os_ + oc) % 2
                                        ].dma_start(
                                            y[t0 + tsub * P:
                                              t0 + (tsub + 1) * P,
                                              ob * OB:(ob + 1) * OB],
                                            ya[:, sl],
                                        )
                                if fine:
                                    continue
                                nc.scalar.activation(
                                    ya[:], yp[:],
                                    mybir.ActivationFunctionType.Silu,
                                    scale=1.0 / W_SCALE,
                                )
                                [nc.sync, nc.scalar][
                                    (tsub * 2 + os_) % 2
                                ].dma_start(
                                    y[t0 + tsub * P:t0 + (tsub + 1) * P,
                                      os_ * half:(os_ + 1) * half],
                                    ya[:],
                                )
                        continue

                    for tsub in range(N_TSUB):
                        store_eng = [nc.gpsimd, nc.sync, nc.scalar][
                            (g * N_TSUB + tsub) % 3
                        ]
                        y_pre = outs.tile([P, D_OUT], F16, tag="ypre")
                        y_act = outs.tile([P, D_OUT], F16, tag="yact")
                        for ob in range(N_OB):
                            ps = psump.tile([P, OB], F32, tag="ps")
                            mm_group(x16, e, tsub, ob, ps[:])
                            # bias add in the x128 domain (bias pre-scaled on
                            # host); fp16 out is exact enough at |v|<~700
                            nc.vector.tensor_tensor(
                                y_pre[:, ob * OB:(ob + 1) * OB], ps[:],
                                b_sb[e][:, ob * OB:(ob + 1) * OB],
                                mybir.AluOpType.add,
                            )
                        # one fused silu per 2048-wide tile amortizes the
                        # ~300ns fixed ACT cost; scale folds the x128
                        # weight scale back out before the nonlinearity
                        nc.scalar.activation(
                            y_act[:], y_pre[:],
                            mybir.ActivationFunctionType.Silu,
                            scale=1.0 / W_SCALE,
                        )
                        store_eng.dma_start(
                            y[t0 + tsub * P:t0 + (tsub + 1) * P, :], y_act[:]
                        )
                    if g == 1:
                        # after g==1 so expert 1's 1MB on the sync ring sits
                        # behind the already-enqueued x1/x2 prefetches
                        load_expert1()

    _split_multi_waits(nc)
    return nc


_NC_CACHE = None


def _get_nc():
    global _NC_CACHE
    if _NC_CACHE is None:
        _NC_CACHE = build_kernel()
    return _NC_CACHE


def _in_maps(sorted_features, routing_matrix, routing_bias):
    maps = []
    for c in range(N_CORES):
        rows = slice(c * TOK_PER_CORE, (c + 1) * TOK_PER_CORE)
        es = slice(c * E_PER_CORE, (c + 1) * E_PER_CORE)
        # [stripe, partition, kb, t]: element (s,p,kb,t) = X_c[s*TS+t, kb*P+p]
        xt_c = np.ascontiguousarray(
            sorted_features[rows]
            .reshape(N_STRIPES, TS, KB, P)
            .transpose(0, 3, 2, 1)
            .astype(NP_F8)
        )
        # pack into the device tile layout [e, p, os, h, oc, kh, o'']:
        # kin = (h*KH+kh)*128 + p, o = os*1024 + oc*512 + o''
        w_c = np.ascontiguousarray(
            (routing_matrix[:, :, es].transpose(2, 0, 1) * W_SCALE)
            .astype(NP_F8)
            .reshape(E_PER_CORE, NWT_G, KH_G, P, 2, 2, D_OUT // 4)
            .transpose(0, 3, 4, 1, 5, 2, 6)
        )
        # bias enters the DVE add in the x128 domain: silu((ps + S*b)/S);
        # fp16 is exact to ~2^-11 relative, far under the fp8 matmul noise
        b_c = np.ascontiguousarray(
            np.broadcast_to(
                (routing_bias[:, es].T * W_SCALE)[:, None, :],
                (E_PER_CORE, P, D_OUT),
            ).astype(np.float16)
        )
        maps.append({"xt": xt_c, "w": w_c, "bb": b_c})
    return maps


def run(sorted_features, routing_matrix, routing_bias, **run_kwargs):
    nc = _get_nc()
    maps = _in_maps(sorted_features, routing_matrix, routing_bias)
    res = run_bass_kernel_spmd(nc, maps, core_ids=list(range(N_CORES)), **run_kwargs)
    out = np.concatenate(
        [res.results[c]["y"].astype(np.float32) for c in range(N_CORES)], axis=0
    )
    return out, res


def kernel(sorted_features, expert_ids_sorted, routing_matrix, routing_bias):
    assert sorted_features.shape == (N_TOKENS, D_IN)
    assert routing_matrix.shape == (D_IN, D_OUT, N_EXPERTS)
    assert routing_bias.shape == (D_OUT, N_EXPERTS)
    out, _ = run(
        np.asarray(sorted_features, dtype=np.float32),
        np.asarray(routing_matrix, dtype=np.float32),
        np.asarray(routing_bias, dtype=np.float32),
    )
    return out

